# revision 29
# baseline (speedup 1.0000x reference)
"""LoLa message-passing kernel for 8 Trainium2 NeuronCores.

Math (algebraically identical to the reference):
  ch0 masses      = f3^2 - f0^2 - f1^2 - f2^2
  ch1 ptsq        = f1^2 + f2^2
  ch2 w_ener@f0, ch4 w_pid@f3, ch5 w_extra0@f4, ch6 w_extra1@f5
  ch3 weighted_d  = 2*(f0*(w_dist@f0) + f1*(w_dist@f1) + f2*(w_dist@f2)
                       - f3*(w_dist@f3))
                    + masses * rowsum(w_dist) + w_dist @ masses

Sharding: model-parallel over particles N (64 output rows per core); combvec
replicated (full contraction operand), weights sliced 1/8 per core.

Device-side design notes:
 - Single-pass bfloat16 matmuls (fp32 PSUM accumulate): the harness gate is
   rel_err < 2e-2 and this lands at ~3.8e-3.
 - Per-core host layout permutes the contraction index so each core's own
   64 rows sit in partitions 0:64 of its first chunk: the row-slice values
   needed by the epilogue (fr) are a partition-slice of the streamed tile,
   and ch0/ch1 fall out of chunk 0's masses intermediates.
 - The features AND weights for each contraction chunk are interleaved in
   ONE DRAM tensor and moved by one 2.2KB-row DMA per chunk on the sync
   HWDGE queue, so chunk c's matmuls are runnable as soon as transfer c
   lands (~1.2us per chunk at the shared-HBM per-core rate).
 - Per-chunk layout (1088 cols): [f0..f5 | dist64 ener64 pid64 x0_64 x1_64]
     MM-A : stat [dist|ener] @ [f0..f3] -> psA  (quad gram + ch2)
     MM-B : stat [pid|x0]    @ [f3|f4]  -> psB  (ch4, ch5)
     MM-C1: stat [x1] (64c)  @ [f5]     -> psC1 (ch6, lands partitions 0:64)
     MM-C2: stat [dist|ener] @ [m|1 1]  -> psC2 (w@masses + rowsum)
 - Dep-free warm-up matmuls keep the PE busy from kernel start (p-state).
 - The Tile scheduler orders each engine's queue from its own latency sim,
   which misjudges the DMA-paced arrival order; tile_wait_until ladders pin
   the DVE/ACT queues to data-arrival order (combines chunk by chunk, then
   quad/stt) so the masses-dependent C2 matmuls fire right behind the data.
 - Outputs: everything except ch3 leaves as soon as its PSUM copies land;
   ch3 (the serial tail) goes out as its own small transfer.
"""

import sys

if "/opt/trn_rl_repo" not in sys.path:
    sys.path.insert(0, "/opt/trn_rl_repo")

import numpy as np
import ml_dtypes

import concourse.bass as bass
import concourse.mybir as mybir
import concourse.tile as tile
from concourse import bacc
from concourse.bass_utils import run_bass_kernel_spmd

B, N, F = 128, 512, 6
NCORES = 8
NS = N // NCORES  # 64 output rows per core
KC = N // 128  # 4 contraction chunks of 128
DT = mybir.dt.float32
BF = mybir.dt.bfloat16
ALU = mybir.AluOpType
ACTF = mybir.ActivationFunctionType

FTW = 768  # ft cols per chunk (6 feats x 128)
WTW = 320  # wt cols per chunk ([dist|ener] 128, [pid|x0] 128, [x1] 64)
CW = FTW + WTW  # fused chunk width (1088)
MOW = 130  # masses cols per chunk (128 masses + 2 ones)
NWARM = 4  # dep-free PE warm-up matmuls (512 cols each)


def _emit(tc, nc, fwt_d, out_d):
    with (
        tc.tile_pool(name="sbuf", bufs=1) as sb,
        tc.tile_pool(name="sqp", bufs=4) as sqp,
        tc.tile_pool(name="mp", bufs=2) as mp,
        tc.tile_pool(name="psum", bufs=1, space="PSUM") as ps,
    ):
        # --- persistent SBUF tiles ---
        fwt = sb.tile([128, KC * CW], BF)  # [c*1088 + (k*128+b | 768+wslot)]
        mos = sb.tile([128, KC * MOW], BF)  # per chunk: [masses | 1 1]
        quad = sb.tile([64, 4 * B], BF)  # bf16: 2x DVE rate, ~0.2% on ch3
        qs = sb.tile([64, 3 * B], BF)
        prime = sb.tile([1, CW], BF)  # DMA-ring wake-up primer
        # out staging, partitions 0:64: olo_e = [ch0 ch1 ch4 ch6], and ch3
        # on its own tile so the late stt2 write shares no tile with the
        # ACT copies (tile-level dep tracking would serialize them)
        olo_e = sb.tile([64, 4 * B], DT)
        olo3 = sb.tile([64, B], DT)
        ohi = sb.tile([128, 2 * B], DT)  # out: ch 2,5 (partitions 64:128)
        warm = sb.tile([128, 5 * B], BF)  # dummy operands for PE warm-up

        # --- PSUM tiles ---
        psA = ps.tile([128, 512], DT)  # [dist|ener] @ [f0|f1|f2|f3]
        psB = ps.tile([128, 256], DT)  # [pid|x0]   @ [f3|f4]
        psC1 = ps.tile([64, B], DT)  # [x1]       @ f5 -> ch6
        psC2 = ps.tile([128, MOW], DT)  # [dist|ener] @ [m|1 1] (rows :64)
        psW = ps.tile([128, 512], DT)  # warm-up sink
        psW2 = ps.tile([128, 512], DT)  # warm-up sink (alternate bank)

        # --- warm-up + constants ---
        nc.vector.memset(warm[:], 0.5)
        mos4 = mos[:].rearrange("p (c x) -> p c x", c=KC, x=MOW)
        nc.gpsimd.memset(mos4[:, :, 128:130], 1.0)

        # dep-free dummy matmuls: PE busy from kernel start (p-state ramp)
        for i in range(NWARM):
            t = psW if i % 2 == 0 else psW2
            nc.tensor.matmul(
                t[:], warm[:, 0:B], warm[:, B: 5 * B], start=True, stop=True
            )

        # --- DMAs in: one fused ft+wt transfer per chunk (2.2KB rows), all
        # on the sync HWDGE queue. Measured: per-core DMA lands ~230GB/s
        # aggregate regardless of queue split or packet size (shared-HBM
        # contention with the other 7 cores); per-chunk transfers give the
        # earliest chunk-0 arrival -> earliest matmul start. ---
        # 1-row primer absorbs the ~1us idle->active DMA ring latency so
        # chunk 0's packets flow as soon as its descriptors land
        nc.sync.dma_start(prime[:], fwt_d[0:1, 0:CW])
        for c in range(KC):
            nc.sync.dma_start(
                fwt[:, c * CW: (c + 1) * CW], fwt_d[:, c * CW: (c + 1) * CW]
            )

        def ftc(c, lo, hi):
            return fwt[:, c * CW + lo: c * CW + hi]

        def wtc(c, lo, hi):
            return fwt[:, c * CW + FTW + lo: c * CW + FTW + hi]

        # scheduler pin: an increasing sim-time floor per step forces each
        # engine queue into data-arrival order (values are far beyond the
        # sim's natural timeline; runtime order is still sem-driven)
        step = [0.02]

        def pin():
            step[0] += 0.0005
            return tc.tile_wait_until(step[0])

        # --- per-chunk squares on ACT ---
        sqs = []
        for c in range(KC):
            sq = sqp.tile([128, 4 * B], BF, name=f"sq{c}")
            with pin():
                nc.scalar.activation(sq[:], ftc(c, 0, 512), ACTF.Square)
            sqs.append(sq)

        # --- masses combines (DVE, bf16): mos[c] = sq3-sq2-sq1-sq0 ---
        def combine(c):
            sq = sqs[c]
            t = mp.tile([128, B], BF, name=f"m{c}")
            with pin():
                nc.vector.tensor_tensor(
                    out=t[:], in0=sq[:, 3 * B: 4 * B], in1=sq[:, 2 * B: 3 * B],
                    op=ALU.subtract,
                )
                nc.vector.tensor_tensor(
                    out=t[:], in0=t[:], in1=sq[:, B: 2 * B], op=ALU.subtract
                )
                nc.vector.tensor_tensor(
                    out=mos[:, c * MOW: c * MOW + B], in0=t[:], in1=sq[:, 0:B],
                    op=ALU.subtract,
                )

        combine(0)
        # ch0 = this core's masses rows; ch1 = ptsq from chunk-0 squares
        with pin():
            nc.vector.tensor_copy(olo_e[:, 0:B], mos[0:64, 0:B])
            nc.vector.tensor_tensor(
                out=olo_e[:, B: 2 * B], in0=sqs[0][0:64, B: 2 * B],
                in1=sqs[0][0:64, 2 * B: 3 * B], op=ALU.add,
            )
        combine(1)
        combine(2)

        # --- matmuls: A/B/C1 per chunk (DMA-gated only); in the last
        # chunk A3 goes first (it gates the quad epilogue), then the
        # already-ready C2 matmuls for chunks 0-2 slot in before B3/C13
        # so psC2 finishes as early as possible ---
        for c in range(3):
            nc.tensor.matmul(
                psA[:], wtc(c, 0, 128), ftc(c, 0, 512),
                start=c == 0, stop=False,
            )
            nc.tensor.matmul(
                psB[:], wtc(c, 128, 256), ftc(c, 384, 640),
                start=c == 0, stop=False,
            )
            nc.tensor.matmul(
                psC1[:], wtc(c, 256, 320), ftc(c, 640, 768),
                start=c == 0, stop=False,
            )
        nc.tensor.matmul(
            psA[:], wtc(3, 0, 128), ftc(3, 0, 512), start=False, stop=True
        )
        for c in range(3):
            with pin():
                nc.tensor.matmul(
                    psC2[:], wtc(c, 0, 128), mos[:, c * MOW: (c + 1) * MOW],
                    start=c == 0, stop=False,
                )
        nc.tensor.matmul(
            psB[:], wtc(3, 128, 256), ftc(3, 384, 640), start=False, stop=True
        )
        nc.tensor.matmul(
            psC1[:], wtc(3, 256, 320), ftc(3, 640, 768), start=False, stop=True
        )

        # --- chunk-3 masses (DVE, ahead of the quad chain) + its C2 ---
        combine(3)
        with pin():
            nc.tensor.matmul(
                psC2[:], wtc(3, 0, 128), mos[:, 3 * MOW: 4 * MOW],
                start=False, stop=True,
            )

        # --- PSUM copies with no tile shared with the DVE epilogue:
        # pinned before it so their emitted waits are just PSUM stops ---
        with pin():
            nc.scalar.copy(olo_e[:, 2 * B: 3 * B], psB[0:64, 0:B])  # ch4
            nc.scalar.copy(olo_e[:, 3 * B: 4 * B], psC1[:, 0:B])  # ch6
            nc.scalar.copy(ohi[64:128, B: 2 * B], psB[64:128, B: 2 * B])  # ch5

        # --- epilogue (DVE): quad = fr * psA, signed sum over k; then
        #     t = masses_R*rowsum + dist@masses ; ch3 = 2*quad_sum + t ---
        # fr = this core's rows of f0..f3 = chunk-0 partitions 0:64
        with pin():
            nc.vector.tensor_tensor(
                out=quad[:], in0=fwt[0:64, 0:512], in1=psA[0:64, :], op=ALU.mult
            )
            nc.vector.tensor_tensor(
                out=qs[:, 0:B], in0=quad[:, 0:B], in1=quad[:, B: 2 * B], op=ALU.add
            )
            nc.vector.tensor_tensor(
                out=qs[:, B: 2 * B], in0=quad[:, 2 * B: 3 * B],
                in1=quad[:, 3 * B: 4 * B], op=ALU.subtract,
            )
            nc.vector.tensor_tensor(
                out=qs[:, 0:B], in0=qs[:, 0:B], in1=qs[:, B: 2 * B], op=ALU.add
            )
        with pin():
            nc.vector.scalar_tensor_tensor(
                out=qs[:, 2 * B: 3 * B],
                in0=olo_e[:, 0:B],
                scalar=psC2[0:64, 128:129],
                in1=psC2[0:64, 0:B],
                op0=ALU.mult,
                op1=ALU.add,
            )
            nc.vector.scalar_tensor_tensor(
                out=olo3[:],
                in0=qs[:, 0:B],
                scalar=2.0,
                in1=qs[:, 2 * B: 3 * B],
                op0=ALU.mult,
                op1=ALU.add,
            )

        # ch2 shares psA with the DVE quad read -> pinned after it
        with pin():
            nc.scalar.copy(ohi[64:128, 0:B], psA[64:128, 0:B])  # ch2 ener@f0

        # --- DMAs out: ch 0,1,4,6 early (sync) + ch 2,5 (scalar); the ch3
        # tail leaves last on the still-active scalar ring ---
        with pin():
            nc.scalar.dma_start(out_d[:, 5 * B: 7 * B], ohi[64:128, :])
            nc.sync.dma_start(out_d[:, 0: 4 * B], olo_e[:])
        with pin():
            nc.scalar.dma_start(out_d[:, 4 * B: 5 * B], olo3[:])


_NC_CACHE = {}


def _get_nc():
    if "nc" not in _NC_CACHE:
        nc = bacc.Bacc(
            "TRN2", target_bir_lowering=False, debug=False, num_devices=NCORES
        )
        fwt_d = nc.dram_tensor("fwt", [128, KC * CW], BF, kind="ExternalInput")
        out_d = nc.dram_tensor("out", [64, 7 * B], DT, kind="ExternalOutput")
        with tile.TileContext(nc) as tc:
            _emit(tc, nc, fwt_d.ap(), out_d.ap())
        nc.compile()
        _NC_CACHE["nc"] = nc
    return _NC_CACHE["nc"]


def make_in_maps(combvec, w_dist, w_ener, w_pid, w_extra0, w_extra1):
    ft_t = np.ascontiguousarray(
        np.transpose(np.asarray(combvec, np.float32), (2, 1, 0))
    )  # (6, 512, 128) [k, m, b]
    ws = [
        np.asarray(w_dist, np.float32),
        np.asarray(w_ener, np.float32),
        np.asarray(w_pid, np.float32),
        np.asarray(w_extra0, np.float32),
        np.asarray(w_extra1, np.float32),
    ]
    in_maps = []
    allm = np.arange(N)
    for core in range(NCORES):
        sl = slice(NS * core, NS * (core + 1))
        # contraction permutation: own 64 rows first (-> chunk0 parts 0:64)
        perm = np.concatenate([allm[sl], np.delete(allm, allm[sl])])
        # ft block: [p, c, k*128 + b] = ft_t[k, perm[c*128+p], b]
        ftp = np.ascontiguousarray(
            ft_t[:, perm, :].reshape(F, KC, 128, B).transpose(2, 1, 0, 3)
        ).reshape(128, KC, FTW)
        # wt block: [p, c, slot] ; W rows sliced to this core's outputs,
        # columns permuted to match the ft contraction order
        wp = [w[sl][:, perm].T.reshape(KC, 128, NS) for w in ws]  # [c, p, n]
        wtp = np.concatenate(wp, axis=2).transpose(1, 0, 2)  # [p, c, 320]
        fwt_np = (
            np.concatenate([ftp, wtp], axis=2)  # [p, c, 1088]
            .reshape(128, KC * CW)
            .astype(ml_dtypes.bfloat16)
        )
        in_maps.append({"fwt": np.ascontiguousarray(fwt_np)})
    return in_maps


# out channel order in the DRAM out tensor columns
OUT_ORDER = [0, 1, 4, 6, 3, 2, 5]


def assemble(results):
    full = np.empty((B, N, 7), np.float32)
    for core, r in enumerate(results):
        o = r["out"].reshape(NS, 7, B)  # (n, slot, b)
        for slot, ch in enumerate(OUT_ORDER):
            full[:, NS * core: NS * (core + 1), ch] = o[:, slot, :].T
    return full


def kernel(combvec, w_dist, w_ener, w_pid, w_extra0, w_extra1, _bench=None):
    in_maps = make_in_maps(combvec, w_dist, w_ener, w_pid, w_extra0, w_extra1)
    nc = _get_nc()
    kw = dict(_bench) if _bench else {}
    res = run_bass_kernel_spmd(nc, in_maps, core_ids=list(range(NCORES)), **kw)
    out = assemble(res.results)
    if _bench is not None:
        kernel.last_results = res
    return out


# revision 30
# speedup vs baseline: 1.1138x; 1.1138x over previous
"""LoLa message-passing kernel for 8 Trainium2 NeuronCores.

Math (algebraically identical to the reference):
  ch0 masses      = f3^2 - f0^2 - f1^2 - f2^2
  ch1 ptsq        = f1^2 + f2^2
  ch2 w_ener@f0, ch4 w_pid@f3, ch5 w_extra0@f4, ch6 w_extra1@f5
  ch3 weighted_d  = 2*(f0*(w_dist@f0) + f1*(w_dist@f1) + f2*(w_dist@f2)
                       - f3*(w_dist@f3))
                    + masses * rowsum(w_dist) + w_dist @ masses

Sharding: model-parallel over particles N (64 output rows per core); combvec
replicated (full contraction operand), weights sliced 1/8 per core.

Device-side design notes:
 - Single-pass bfloat16 matmuls (fp32 PSUM accumulate): the harness gate is
   rel_err < 2e-2 and this lands at ~3.8e-3.
 - Per-core host layout permutes the contraction index so each core's own
   64 rows sit in partitions 0:64 of its first chunk: the row-slice values
   needed by the epilogue (fr) are a partition-slice of the streamed tile,
   and ch0/ch1 fall out of chunk 0's masses intermediates.
 - The features AND weights for each contraction chunk are interleaved in
   ONE DRAM tensor and moved by one 2.2KB-row DMA per chunk on the sync
   HWDGE queue, so chunk c's matmuls are runnable as soon as transfer c
   lands (~1.2us per chunk at the shared-HBM per-core rate).
 - Per-chunk layout (1088 cols): [f0..f5 | dist64 ener64 pid64 x0_64 x1_64]
     MM-A : stat [dist|ener] @ [f0..f3] -> psA  (quad gram + ch2)
     MM-B : stat [pid|x0]    @ [f3|f4]  -> psB  (ch4, ch5)
     MM-C1: stat [x1] (64c)  @ [f5]     -> psC1 (ch6, lands partitions 0:64)
     MM-C2: stat [dist|ener] @ [m|1 1]  -> psC2 (w@masses + rowsum)
 - Dep-free warm-up matmuls keep the PE busy from kernel start (p-state).
 - The Tile scheduler orders each engine's queue from its own latency sim,
   which misjudges the DMA-paced arrival order; tile_wait_until ladders pin
   the DVE/ACT queues to data-arrival order (combines chunk by chunk, then
   quad/stt) so the masses-dependent C2 matmuls fire right behind the data.
 - Outputs: everything except ch3 leaves as soon as its PSUM copies land;
   ch3 (the serial tail) goes out as its own small transfer.
"""

import sys

if "/opt/trn_rl_repo" not in sys.path:
    sys.path.insert(0, "/opt/trn_rl_repo")

import numpy as np
import ml_dtypes

import concourse.bass as bass
import concourse.mybir as mybir
import concourse.tile as tile
from concourse import bacc
from concourse.bass_utils import run_bass_kernel_spmd

B, N, F = 128, 512, 6
NCORES = 8
NS = N // NCORES  # 64 output rows per core
KC = N // 128  # 4 contraction chunks of 128
DT = mybir.dt.float32
BF = mybir.dt.bfloat16
ALU = mybir.AluOpType
ACTF = mybir.ActivationFunctionType

FTW = 768  # ft cols per chunk (6 feats x 128)
WTW = 320  # wt cols per chunk ([dist|ener] 128, [pid|x0] 128, [x1] 64)
CW = FTW + WTW  # fused chunk width (1088)
MOW = 130  # masses cols per chunk (128 masses + 2 ones)
NWARM = 4  # dep-free PE warm-up matmuls (512 cols each)


def _emit(tc, nc, fwt_d, out_d):
    with (
        tc.tile_pool(name="sbuf", bufs=1) as sb,
        tc.tile_pool(name="sqp", bufs=4) as sqp,
        tc.tile_pool(name="mp", bufs=2) as mp,
        tc.tile_pool(name="psum", bufs=1, space="PSUM") as ps,
    ):
        # --- persistent SBUF tiles ---
        fwt = sb.tile([128, KC * CW], BF)  # [c*1088 + (k*128+b | 768+wslot)]
        mos = sb.tile([128, KC * MOW], BF)  # per chunk: [masses | 1 1]
        quad = sb.tile([64, 4 * B], BF)  # bf16: 2x DVE rate, ~0.2% on ch3
        qs = sb.tile([64, 3 * B], BF)
        # out staging, partitions 0:64: olo_e = [ch0 ch1 ch4 ch6], and ch3
        # on its own tile so the late stt2 write shares no tile with the
        # ACT copies (tile-level dep tracking would serialize them)
        olo_e = sb.tile([64, 4 * B], DT)
        olo3 = sb.tile([64, B], DT)
        ohi = sb.tile([128, 2 * B], DT)  # out: ch 2,5 (partitions 64:128)
        warm = sb.tile([128, 5 * B], BF)  # dummy operands for PE warm-up

        # --- PSUM tiles ---
        psA = ps.tile([128, 512], DT)  # [dist|ener] @ [f0|f1|f2|f3]
        psB = ps.tile([128, 256], DT)  # [pid|x0]   @ [f3|f4]
        psC1 = ps.tile([64, B], DT)  # [x1]       @ f5 -> ch6
        psC2 = ps.tile([128, MOW], DT)  # [dist|ener] @ [m|1 1] (rows :64)
        psW = ps.tile([128, 512], DT)  # warm-up sink
        psW2 = ps.tile([128, 512], DT)  # warm-up sink (alternate bank)

        # --- warm-up + constants ---
        nc.vector.memset(warm[:], 0.5)
        mos4 = mos[:].rearrange("p (c x) -> p c x", c=KC, x=MOW)
        nc.gpsimd.memset(mos4[:, :, 128:130], 1.0)

        # dep-free dummy matmuls: PE busy from kernel start (p-state ramp)
        for i in range(NWARM):
            t = psW if i % 2 == 0 else psW2
            nc.tensor.matmul(
                t[:], warm[:, 0:B], warm[:, B: 5 * B], start=True, stop=True
            )

        # --- DMAs in: one fused ft+wt transfer per chunk (2.2KB rows), all
        # on the sync HWDGE queue. Measured: per-core DMA lands ~230GB/s
        # aggregate regardless of queue split or packet size (shared-HBM
        # contention with the other 7 cores); per-chunk transfers give the
        # earliest chunk-0 arrival -> earliest matmul start. ---
        for c in range(KC):
            nc.sync.dma_start(
                fwt[:, c * CW: (c + 1) * CW], fwt_d[:, c * CW: (c + 1) * CW]
            )

        def ftc(c, lo, hi):
            return fwt[:, c * CW + lo: c * CW + hi]

        def wtc(c, lo, hi):
            return fwt[:, c * CW + FTW + lo: c * CW + FTW + hi]

        # scheduler pin: an increasing sim-time floor per step forces each
        # engine queue into data-arrival order (values are far beyond the
        # sim's natural timeline; runtime order is still sem-driven)
        step = [0.02]

        def pin():
            step[0] += 0.0005
            return tc.tile_wait_until(step[0])

        # --- per-chunk squares on ACT ---
        sqs = []
        for c in range(KC):
            sq = sqp.tile([128, 4 * B], BF, name=f"sq{c}")
            with pin():
                nc.scalar.activation(sq[:], ftc(c, 0, 512), ACTF.Square)
            sqs.append(sq)

        # --- masses combines (DVE, bf16): mos[c] = sq3-sq2-sq1-sq0 ---
        def combine(c):
            sq = sqs[c]
            t = mp.tile([128, B], BF, name=f"m{c}")
            with pin():
                nc.vector.tensor_tensor(
                    out=t[:], in0=sq[:, 3 * B: 4 * B], in1=sq[:, 2 * B: 3 * B],
                    op=ALU.subtract,
                )
                nc.vector.tensor_tensor(
                    out=t[:], in0=t[:], in1=sq[:, B: 2 * B], op=ALU.subtract
                )
                nc.vector.tensor_tensor(
                    out=mos[:, c * MOW: c * MOW + B], in0=t[:], in1=sq[:, 0:B],
                    op=ALU.subtract,
                )

        combine(0)
        # ch0 = this core's masses rows; ch1 = ptsq from chunk-0 squares
        with pin():
            nc.vector.tensor_copy(olo_e[:, 0:B], mos[0:64, 0:B])
            nc.vector.tensor_tensor(
                out=olo_e[:, B: 2 * B], in0=sqs[0][0:64, B: 2 * B],
                in1=sqs[0][0:64, 2 * B: 3 * B], op=ALU.add,
            )
        combine(1)
        combine(2)

        # --- matmuls: A/B/C1 per chunk (DMA-gated only) ---
        for c in range(KC):
            nc.tensor.matmul(
                psA[:], wtc(c, 0, 128), ftc(c, 0, 512),
                start=c == 0, stop=c == KC - 1,
            )
            nc.tensor.matmul(
                psB[:], wtc(c, 128, 256), ftc(c, 384, 640),
                start=c == 0, stop=c == KC - 1,
            )
            nc.tensor.matmul(
                psC1[:], wtc(c, 256, 320), ftc(c, 640, 768),
                start=c == 0, stop=c == KC - 1,
            )

        # --- masses-dependent C2 matmuls, chunks 0-2 ---
        for c in range(3):
            with pin():
                nc.tensor.matmul(
                    psC2[:], wtc(c, 0, 128), mos[:, c * MOW: (c + 1) * MOW],
                    start=c == 0, stop=False,
                )

        # --- chunk-3 masses (DVE, ahead of the quad chain) + its C2 ---
        combine(3)
        with pin():
            nc.tensor.matmul(
                psC2[:], wtc(3, 0, 128), mos[:, 3 * MOW: 4 * MOW],
                start=False, stop=True,
            )

        # --- PSUM copies with no tile shared with the DVE epilogue:
        # pinned before it so their emitted waits are just PSUM stops ---
        with pin():
            nc.scalar.copy(olo_e[:, 2 * B: 3 * B], psB[0:64, 0:B])  # ch4
            nc.scalar.copy(olo_e[:, 3 * B: 4 * B], psC1[:, 0:B])  # ch6
            nc.scalar.copy(ohi[64:128, B: 2 * B], psB[64:128, B: 2 * B])  # ch5

        # --- epilogue (DVE): quad = fr * psA, signed sum over k; then
        #     t = masses_R*rowsum + dist@masses ; ch3 = 2*quad_sum + t ---
        # fr = this core's rows of f0..f3 = chunk-0 partitions 0:64
        with pin():
            nc.vector.tensor_tensor(
                out=quad[:], in0=fwt[0:64, 0:512], in1=psA[0:64, :], op=ALU.mult
            )
            nc.vector.tensor_tensor(
                out=qs[:, 0:B], in0=quad[:, 0:B], in1=quad[:, B: 2 * B], op=ALU.add
            )
            nc.vector.tensor_tensor(
                out=qs[:, B: 2 * B], in0=quad[:, 2 * B: 3 * B],
                in1=quad[:, 3 * B: 4 * B], op=ALU.subtract,
            )
            nc.vector.tensor_tensor(
                out=qs[:, 0:B], in0=qs[:, 0:B], in1=qs[:, B: 2 * B], op=ALU.add
            )
        with pin():
            nc.vector.scalar_tensor_tensor(
                out=qs[:, 2 * B: 3 * B],
                in0=olo_e[:, 0:B],
                scalar=psC2[0:64, 128:129],
                in1=psC2[0:64, 0:B],
                op0=ALU.mult,
                op1=ALU.add,
            )
            nc.vector.scalar_tensor_tensor(
                out=olo3[:],
                in0=qs[:, 0:B],
                scalar=2.0,
                in1=qs[:, 2 * B: 3 * B],
                op0=ALU.mult,
                op1=ALU.add,
            )

        # ch2 shares psA with the DVE quad read -> pinned after it
        with pin():
            nc.scalar.copy(ohi[64:128, 0:B], psA[64:128, 0:B])  # ch2 ener@f0

        # --- DMAs out: ch 0,1,4,6 early (sync) + ch 2,5 (scalar); the ch3
        # tail leaves last on the still-active scalar ring ---
        with pin():
            nc.scalar.dma_start(out_d[:, 5 * B: 7 * B], ohi[64:128, :])
            nc.sync.dma_start(out_d[:, 0: 4 * B], olo_e[:])
        with pin():
            nc.scalar.dma_start(out_d[:, 4 * B: 5 * B], olo3[:])


_NC_CACHE = {}


def _get_nc():
    if "nc" not in _NC_CACHE:
        nc = bacc.Bacc(
            "TRN2", target_bir_lowering=False, debug=False, num_devices=NCORES
        )
        fwt_d = nc.dram_tensor("fwt", [128, KC * CW], BF, kind="ExternalInput")
        out_d = nc.dram_tensor("out", [64, 7 * B], DT, kind="ExternalOutput")
        with tile.TileContext(nc) as tc:
            _emit(tc, nc, fwt_d.ap(), out_d.ap())
        nc.compile()
        _NC_CACHE["nc"] = nc
    return _NC_CACHE["nc"]


def make_in_maps(combvec, w_dist, w_ener, w_pid, w_extra0, w_extra1):
    ft_t = np.ascontiguousarray(
        np.transpose(np.asarray(combvec, np.float32), (2, 1, 0))
    )  # (6, 512, 128) [k, m, b]
    ws = [
        np.asarray(w_dist, np.float32),
        np.asarray(w_ener, np.float32),
        np.asarray(w_pid, np.float32),
        np.asarray(w_extra0, np.float32),
        np.asarray(w_extra1, np.float32),
    ]
    in_maps = []
    allm = np.arange(N)
    for core in range(NCORES):
        sl = slice(NS * core, NS * (core + 1))
        # contraction permutation: own 64 rows first (-> chunk0 parts 0:64)
        perm = np.concatenate([allm[sl], np.delete(allm, allm[sl])])
        # ft block: [p, c, k*128 + b] = ft_t[k, perm[c*128+p], b]
        ftp = np.ascontiguousarray(
            ft_t[:, perm, :].reshape(F, KC, 128, B).transpose(2, 1, 0, 3)
        ).reshape(128, KC, FTW)
        # wt block: [p, c, slot] ; W rows sliced to this core's outputs,
        # columns permuted to match the ft contraction order
        wp = [w[sl][:, perm].T.reshape(KC, 128, NS) for w in ws]  # [c, p, n]
        wtp = np.concatenate(wp, axis=2).transpose(1, 0, 2)  # [p, c, 320]
        fwt_np = (
            np.concatenate([ftp, wtp], axis=2)  # [p, c, 1088]
            .reshape(128, KC * CW)
            .astype(ml_dtypes.bfloat16)
        )
        in_maps.append({"fwt": np.ascontiguousarray(fwt_np)})
    return in_maps


# out channel order in the DRAM out tensor columns
OUT_ORDER = [0, 1, 4, 6, 3, 2, 5]


def assemble(results):
    full = np.empty((B, N, 7), np.float32)
    for core, r in enumerate(results):
        o = r["out"].reshape(NS, 7, B)  # (n, slot, b)
        for slot, ch in enumerate(OUT_ORDER):
            full[:, NS * core: NS * (core + 1), ch] = o[:, slot, :].T
    return full


def kernel(combvec, w_dist, w_ener, w_pid, w_extra0, w_extra1, _bench=None):
    in_maps = make_in_maps(combvec, w_dist, w_ener, w_pid, w_extra0, w_extra1)
    nc = _get_nc()
    kw = dict(_bench) if _bench else {}
    res = run_bass_kernel_spmd(nc, in_maps, core_ids=list(range(NCORES)), **kw)
    out = assemble(res.results)
    if _bench is not None:
        kernel.last_results = res
    return out
